# revision 1
# baseline (speedup 1.0000x reference)
"""Multi-head self-attention with RoPE — Trainium2 Bass kernel, 8 NeuronCores.

Sharding: core c = 2*b + g handles batch b = c//2 and head-group g = c%2
(8 of the 16 heads).  Within each batch pair the cores AllGather their
normalized attention outputs (O^T, bf16) and both run the full output
projection redundantly; the host keeps the even core's y.  No reduction
on the host.

Per-core dataflow (matmuls bf16, fp32 PSUM accumulation):
  xT [E, L] bf16 (pre-transposed on host)
  QKV:   Q^T/K^T pair tiles via W-stationary matmuls; V natural [L, 512].
  RoPE:  weights pre-permuted on host to de-interleave even/odd dims, so
         rotate-half becomes a 32-partition block swap (SBUF->SBUF DMA);
         cos/sin multiplies on GPSIMD, combine add on DVE.
  Scores:S^T half-tiles [Lk=128, Lq=512] per head, heads row-tiled on PE;
         three halves share a [128, 1536] PSUM tile (double buffered) so
         exp runs as few, wide ACT instructions overlapped with PE.
  Softmax: denominator via ones-column appended to V (PSUM partition 64
         of O^T); reciprocal_approx + gpsimd partition_broadcast.
  AV:    O^T[65, 512] += V_aug^T A^T over 16 Lk chunks.
  Proj:  y[lq] = Ocat^T.T @ w_out, fp32 [L, E].
"""

import contextlib
import functools

import numpy as np
import ml_dtypes

import concourse.bass as bass
import concourse.mybir as mybir
import concourse.tile as tile
from concourse import bacc
from concourse.bass_utils import run_bass_kernel_spmd

BF16 = mybir.dt.bfloat16
F32 = mybir.dt.float32
N_CORES = 8
ROPE_THETA = 10000.0

B_FULL, L_FULL, E_FULL = 4, 2048, 1024
H_FULL = 16


def _emit(tc, nc, xT, wqkv, wout, cosT, sinT, y, L, E, HC, D, taps=None, use_collective=True):
    P = 128
    EC = E // P                 # E chunks of 128 (contraction)
    NPAIR = HC // 2             # head pairs per core
    LT = L // 512               # 512-wide L tiles
    LKC = L // P                # 128-wide Lk chunks
    A = HC * D                  # local attention width (512)
    scale = 1.0 / float(np.sqrt(D))
    Exp = mybir.ActivationFunctionType.Exp

    ctx = contextlib.ExitStack()
    pool = ctx.enter_context(tc.tile_pool(name="sb", bufs=1))
    psum = ctx.enter_context(tc.tile_pool(name="ps", bufs=1, space="PSUM"))
    work = ctx.enter_context(tc.tile_pool(name="wk", bufs=1))
    dram = ctx.enter_context(tc.tile_pool(name="dr", bufs=1, space="DRAM"))

    # ---- persistent SBUF buffers ----
    xt_sb = pool.tile([P, EC, L], BF16, tag="xbuf")
    wqkv_sb = pool.tile([P, EC, 3 * A], BF16, tag="wqkv")
    wout_sb = pool.tile([P, EC, E], BF16, tag="wout")
    cos_sb = pool.tile([P, L], BF16, tag="costab")
    sin_sb = pool.tile([P, L], BF16, tag="sintab")
    qk_sb = pool.tile([P, 2, NPAIR, L], BF16, tag="qk")      # [pair-rows, q/k, pair, L]
    vaug_sb = pool.tile([P, LKC, HC, D + 1], BF16, tag="vaug")
    ot_sb = pool.tile([64, HC, L], BF16, tag="ot")           # normalized O^T per head

    nc.sync.dma_start(xt_sb[:], xT.ap().rearrange("(c p) l -> p c l", p=P))
    nc.sync.dma_start(wqkv_sb[:], wqkv.ap().rearrange("(c p) n -> p c n", p=P))
    nc.sync.dma_start(wout_sb[:], wout.ap().rearrange("(c p) n -> p c n", p=P))
    nc.sync.dma_start(cos_sb[:], cosT.ap())
    nc.sync.dma_start(sin_sb[:], sinT.ap())

    # ones column for the softmax denominator
    nc.vector.memset(vaug_sb[:, :, :, D : D + 1], 1.0)

    # ---- V = x @ Wv, natural [L, A] layout, 2 L-chunks per PSUM tile ----
    for vg in range(LKC // 2):
        ps = psum.tile([P, 1024], F32, tag="sc", bufs=2)
        for i in range(2):
            lt = vg * 2 + i
            for e in range(EC):
                nc.tensor.matmul(
                    ps[:, i * 512 : (i + 1) * 512],
                    lhsT=xt_sb[:, e, lt * P : (lt + 1) * P],
                    rhs=wqkv_sb[:, e, 2 * A : 3 * A],
                    start=(e == 0),
                    stop=(e == EC - 1),
                )
        nc.scalar.copy(
            out=vaug_sb[:, vg * 2 : (vg + 1) * 2, :, 0:D],
            in_=ps[:].rearrange("p (t h d) -> p t h d", h=HC, d=D),
        )

    # ---- Q^T / K^T + RoPE ----
    # psum tile cols: [q | k] for one 512-wide L tile
    for p in range(NPAIR):
        for lt in range(LT):
            ps = psum.tile([P, 1024], F32, tag="sc", bufs=2)
            for qk in range(2):
                wcol = qk * A + p * P
                for e in range(EC):
                    nc.tensor.matmul(
                        ps[:, qk * 512 : (qk + 1) * 512],
                        lhsT=wqkv_sb[:, e, wcol : wcol + P],
                        rhs=xt_sb[:, e, lt * 512 : (lt + 1) * 512],
                        start=(e == 0),
                        stop=(e == EC - 1),
                    )
            Lsl = slice(lt * 512, (lt + 1) * 512)
            tab = lambda sb: (
                sb[:, Lsl][:, None, :].to_broadcast([P, 2, 512])
            )
            qs = work.tile([P, 1024], BF16, tag="qs", bufs=3)
            nc.scalar.copy(out=qs[:], in_=ps[:])
            qs_v = qs[:].rearrange("p (q c) -> p q c", q=2)
            w = work.tile([P, 1024], BF16, tag="w", bufs=3)
            t = work.tile([P, 1024], BF16, tag="w", bufs=3)
            nc.gpsimd.tensor_mul(w[:].rearrange("p (q c) -> p q c", q=2), qs_v, tab(sin_sb))
            nc.gpsimd.tensor_mul(t[:].rearrange("p (q c) -> p q c", q=2), qs_v, tab(cos_sb))
            wsw = work.tile([P, 1024], BF16, tag="wsw", bufs=2)
            for blk in range(4):
                sb = blk ^ 1  # swap 32-row blocks pairwise
                nc.sync.dma_start(
                    wsw[blk * 32 : (blk + 1) * 32, :], w[sb * 32 : (sb + 1) * 32, :]
                )
            out_ap = qk_sb[:, :, p, Lsl]  # [P, 2, 512]
            nc.vector.tensor_add(
                out_ap,
                t[:].rearrange("p (q c) -> p q c", q=2),
                wsw[:].rearrange("p (q c) -> p q c", q=2),
            )

    # ---- attention + AllGather staging ----
    cc_half = NPAIR // 2 * P  # feature rows per collective (2 pairs x 128)
    cc_in = [
        dram.tile([cc_half, L], BF16, tag=f"ccin{i}", bufs=1, name=f"ccin{i}")
        for i in range(2)
    ]
    cc_out = [
        dram.tile([2, cc_half, L], BF16, tag=f"ccout{i}", bufs=1, name=f"ccout{i}")
        for i in range(2)
    ]

    for p in range(NPAIR):
        h0, h1 = 2 * p, 2 * p + 1
        for lq in range(LT):
            ot0 = psum.tile([65, 512], F32, tag="ot", bufs=2)
            ot1 = psum.tile([65, 512], F32, tag="ot", bufs=2)
            ots = (ot0, ot1)
            Lq = slice(lq * 512, (lq + 1) * 512)
            # halves: (head, lk) pairs in lk-major order, grouped 3 per
            # [128, 1536] psum tile so exp runs as wide ACT instructions.
            halves = [(hh, lk) for lk in range(LKC) for hh in range(2)]
            gi = 0
            while gi < len(halves):
                grp = halves[gi : gi + 3]
                nh = len(grp)
                ps = psum.tile([P, 1536], F32, tag="sc", bufs=2)
                for j, (hh, lk) in enumerate(grp):
                    nc.tensor.matmul(
                        ps[:, j * 512 : (j + 1) * 512],
                        lhsT=qk_sb[hh * 64 : (hh + 1) * 64, 1, p, lk * P : (lk + 1) * P],
                        rhs=qk_sb[hh * 64 : (hh + 1) * 64, 0, p, Lq],
                        start=True,
                        stop=True,
                    )
                at = work.tile([P, 1536], BF16, tag="at", bufs=4)
                nc.scalar.activation(at[:, : nh * 512], ps[:, : nh * 512], Exp, scale=scale)
                for j, (hh, lk) in enumerate(grp):
                    nc.tensor.matmul(
                        ots[hh][:],
                        lhsT=vaug_sb[:, lk, 2 * p + hh, :],
                        rhs=at[:, j * 512 : (j + 1) * 512],
                        start=(lk == 0),
                        stop=(lk == LKC - 1),
                    )
                gi += nh
            for hh, otp in ((0, ot0), (1, ot1)):
                # denominator: PSUM row 64 -> SBUF row 64 -> (DMA) row 0 ->
                # reciprocal -> broadcast to 64 partitions -> scale O^T.
                den = work.tile([65, 512], F32, tag="den", bufs=1)
                nc.vector.tensor_copy(out=den[64:65, :], in_=otp[64:65, :])
                den0 = work.tile([1, 512], F32, tag="den0", bufs=1)
                nc.sync.dma_start(den0[0:1, :], den[64:65, :])
                rec0 = work.tile([1, 512], F32, tag="rec0", bufs=1)
                nc.vector.reciprocal_approx_fast(rec0[0:1, :], den0[0:1, :])
                rbc = work.tile([64, 512], F32, tag="rbc", bufs=2)
                nc.gpsimd.partition_broadcast(rbc[:], rec0[0:1, :])
                nc.vector.tensor_mul(ot_sb[:, 2 * p + hh, Lq], otp[0:64, :], rbc[:])
        if p % 2 == 1:
            half = p // 2
            src = ot_sb[:, half * (NPAIR // 2) * 2 : (half + 1) * (NPAIR // 2) * 2, :]
            nc.sync.dma_start(
                cc_in[half][:].rearrange("(h d) l -> d h l", d=64),
                src,
            )
            if use_collective:
                nc.gpsimd.collective_compute(
                    "AllGather",
                    mybir.AluOpType.bypass,
                    replica_groups=[[2 * i, 2 * i + 1] for i in range(N_CORES // 2)],
                    ins=[cc_in[half][:].opt()],
                    outs=[cc_out[half][:].opt()],
                )
            else:  # timing-analysis build: stand-in DMAs, no collective
                nc.sync.dma_start(cc_out[half][0], cc_in[half][:])
                nc.sync.dma_start(cc_out[half][1], cc_in[half][:])

    # ---- gather Ocat^T into SBUF: [128, EC, L], global feature-major ----
    ocat_sb = pool.tile([P, EC, L], BF16, tag="xbuf")
    for g2 in range(2):
        for half in range(2):
            blk = cc_out[half][g2]  # [cc_half, L]
            for q in range(cc_half // P):
                f = g2 * (2 * cc_half) + half * cc_half + q * P  # global row
                nc.sync.dma_start(ocat_sb[:, f // P, :], blk[q * P : (q + 1) * P, :])

    if taps is not None:
        nc.sync.dma_start(taps["ot"].ap(), ot_sb[:])
        nc.sync.dma_start(
            taps["ocat"].ap().rearrange("(c p) l -> p c l", p=P), ocat_sb[:]
        )
        nc.sync.dma_start(taps["qk"].ap(), qk_sb[:])
        nc.sync.dma_start(taps["vaug"].ap(), vaug_sb[:])

    # ---- output projection: y[lq] = Ocat^T.T @ wout (full L, redundant) ----
    for lq in range(L // P):
        ps = psum.tile([P, 1024], F32, tag="sc", bufs=2)
        for nhf in range(E // 512):
            for e in range(EC):
                nc.tensor.matmul(
                    ps[:, nhf * 512 : (nhf + 1) * 512],
                    lhsT=ocat_sb[:, e, lq * P : (lq + 1) * P],
                    rhs=wout_sb[:, e, nhf * 512 : (nhf + 1) * 512],
                    start=(e == 0),
                    stop=(e == EC - 1),
                )
        yt = work.tile([P, E], F32, tag="yt", bufs=2)
        nc.scalar.copy(out=yt[:], in_=ps[:, :E])
        nc.sync.dma_start(y.ap()[lq * P : (lq + 1) * P, :], yt[:])

    ctx.close()


@functools.lru_cache(maxsize=2)
def build_module(L=L_FULL, E=E_FULL, HC=H_FULL // 2, D=64, asserts=False,
                 debug_taps=False, use_collective=True):
    nc = bacc.Bacc(
        "TRN2",
        target_bir_lowering=False,
        debug=False,
        enable_asserts=asserts,
        num_devices=N_CORES,
    )
    A = HC * D
    xT = nc.dram_tensor("xT", [E, L], BF16, kind="ExternalInput")
    wqkv = nc.dram_tensor("wqkv", [E, 3 * A], BF16, kind="ExternalInput")
    wout = nc.dram_tensor("wout", [E, E], BF16, kind="ExternalInput")
    cosT = nc.dram_tensor("cosT", [128, L], BF16, kind="ExternalInput")
    sinT = nc.dram_tensor("sinT", [128, L], BF16, kind="ExternalInput")
    y = nc.dram_tensor("y", [L, E], F32, kind="ExternalOutput")
    taps = None
    if debug_taps:
        taps = {
            "ot": nc.dram_tensor("ot_dbg", [64, HC, L], BF16, kind="ExternalOutput"),
            "ocat": nc.dram_tensor("ocat_dbg", [E, L], BF16, kind="ExternalOutput"),
            "qk": nc.dram_tensor("qk_dbg", [128, 2, HC // 2, L], BF16, kind="ExternalOutput"),
            "vaug": nc.dram_tensor(
                "vaug_dbg", [128, L // 128, HC, D + 1], BF16, kind="ExternalOutput"
            ),
        }
    with tile.TileContext(nc) as tc:
        _emit(tc, nc, xT, wqkv, wout, cosT, sinT, y, L, E, HC, D, taps=taps,
              use_collective=use_collective)
    nc.compile()
    return nc


def _rope_tables(L, D):
    """cos/sin tables in the de-interleaved 32-row layout, stacked x4.

    Row p (p in [0,32)): frequency p (covers original dims 2p / 2p+1).
    sin is pre-signed for the post-swap add: blocks [+s, -s, +s, -s].
    """
    half = D // 2
    inv_freq = 1.0 / (ROPE_THETA ** (np.arange(0, D, 2, dtype=np.float64) / D))
    freqs = np.arange(L, dtype=np.float64)[None, :] * inv_freq[:, None]  # [32, L]
    cos32 = np.cos(freqs)
    sin32 = np.sin(freqs)
    bf = ml_dtypes.bfloat16
    cos = np.tile(cos32, (128 // half, 1)).astype(bf)
    sin_block = np.concatenate([sin32, -sin32], axis=0)  # [64, L]
    sin = np.tile(sin_block, (2, 1)).astype(bf)
    return cos, sin


def _deint_cols(base, h, D):
    """Column indices of head h (offset base), even dims then odd dims."""
    cols = base + h * D + np.arange(D)
    return np.concatenate([cols[0::2], cols[1::2]])


def make_core_inputs(x, w_qkv, w_out, H=H_FULL, D=64):
    """Per-core input dicts from the full (unsharded) fp32 inputs."""
    Bv, L, E = x.shape
    HC = H // (N_CORES // Bv)
    A_full = H * D
    bf = ml_dtypes.bfloat16
    cos, sin = _rope_tables(L, D)
    wout_bf = np.ascontiguousarray(w_out).astype(bf)
    in_maps = []
    for c in range(N_CORES):
        b, g = c // 2, c % 2
        xT = np.ascontiguousarray(x[b].T).astype(bf)
        qcols = []
        kcols = []
        vcols = []
        for p in range(HC // 2):
            for hh in range(2):
                h = g * HC + 2 * p + hh
                qcols.append(_deint_cols(0, h, D))
                kcols.append(_deint_cols(A_full, h, D))
        for hl in range(HC):
            h = g * HC + hl
            vcols.append(2 * A_full + h * D + np.arange(D))
        cols = np.concatenate(qcols + kcols + vcols)
        wqkv_c = np.ascontiguousarray(w_qkv[:, cols]).astype(bf)
        in_maps.append(
            {
                "xT": xT,
                "wqkv": wqkv_c,
                "wout": wout_bf,
                "cosT": cos[:, :L].copy(),
                "sinT": sin[:, :L].copy(),
            }
        )
    return in_maps


def kernel(x, w_qkv, w_out):
    x = np.asarray(x)
    w_qkv = np.asarray(w_qkv)
    w_out = np.asarray(w_out)
    Bv, L, E = x.shape
    nc = build_module(L=L, E=E)
    in_maps = make_core_inputs(x, w_qkv, w_out)
    res = run_bass_kernel_spmd(nc, in_maps, core_ids=list(range(N_CORES)))
    out = np.empty((Bv, L, E), dtype=np.float32)
    for b in range(Bv):
        out[b] = res.results[2 * b]["y"]
    return out



# revision 16
# speedup vs baseline: 1.2079x; 1.2079x over previous
"""Multi-head self-attention with RoPE — Trainium2 Bass kernel, 8 NeuronCores.

Sharding: core c = 2*b + g handles batch b = c//2 and head-group g = c%2
(8 of the 16 heads).  Each core computes attention for its 8 heads over the
full sequence, then the pair exchanges *unnormalized* attention outputs
(O^T rows 1..64) together with reciprocal softmax denominators (row 0) via
two AllToAll collectives; each core normalizes the received halves and runs
the output projection for its own L-half only (even core: rows [0, L/2),
odd core: rows [L/2, L)).  The host concatenates the two halves.

Per-core dataflow (matmuls bf16, fp32 PSUM accumulation):
  xT [E, L] bf16 (pre-transposed on host)
  QKV:   Q^T/K^T via W-stationary matmuls, RoPE fused in (de-interleaved
         even/odd dims so rotate-half is a 32-partition block swap via
         SBUF->SBUF DMA; muls on GPSIMD, combine add on DVE).  V natural.
  Sched: QK(pair0) + V(heads 0..3) run up front; QK(pair p+1) and the rest
         of V are spliced between attention groups so the PE never starves
         while ScalarE (the exp bottleneck, ~255us) runs continuously.
  Scores:S^T [Lk=128, Lq=512] per head; the two heads of a pair sit at
         PSUM cols [0,512) / [512,1024) with lhsT at base partitions 0/64,
         so the 64-contraction matmuls row-tile and run concurrently.
  Softmax: exp as one wide ACT instruction per [128, 1024] PSUM tile; the
         denominator rides along as a leading ones-row in V_aug = [1 | V].
  AV:    O^T[65, 512] += V_aug^T A^T over Lk chunks; row 0 = denominator.
  Evac:  reciprocal of the denominator + raw O rows copied to SBUF (fast,
         keeps PSUM free and the PE warm), staged per lq tile to DRAM.
  Exchange: AllToAll over each core pair: chunk 0 = my lq tiles for the
         even core, chunk 1 = for the odd core.
  Norm:  receiver-side: rbc = ones^T @ recip-row (PE broadcast matmul),
         O_norm = O * rbc (DVE), assembled into feature-major Ocat.
  Proj:  y[half] = Ocat^T.T @ w_out for this core's L-half only.
"""

import contextlib
import functools
from collections import deque

import numpy as np
import ml_dtypes

import concourse.bass as bass
import concourse.mybir as mybir
import concourse.tile as tile
from concourse import bacc
from concourse.bass_utils import run_bass_kernel_spmd

BF16 = mybir.dt.bfloat16
F32 = mybir.dt.float32
N_CORES = 8
ROPE_THETA = 10000.0

B_FULL, L_FULL, E_FULL = 4, 2048, 1024
H_FULL = 16


def _emit(tc, nc, xT, wqkv, wout, cosT, sinT, y, L, E, HC, D, use_collective=True,
          taps=None):
    P = 128
    EC = E // P                 # E chunks of 128 (contraction)
    NPAIR = HC // 2             # head pairs per core (4)
    LT = L // 512               # 512-wide L (query) tiles
    TH = LT // 2                # lq tiles per core-half
    LKC = L // P                # 128-wide Lk chunks
    A = HC * D                  # local attention width (512)
    scale = 1.0 / float(np.sqrt(D))
    Exp = mybir.ActivationFunctionType.Exp

    ctx = contextlib.ExitStack()
    pool = ctx.enter_context(tc.tile_pool(name="sb", bufs=1))
    psum = ctx.enter_context(tc.tile_pool(name="ps", bufs=1, space="PSUM"))
    work = ctx.enter_context(tc.tile_pool(name="wk", bufs=1))
    dram = ctx.enter_context(tc.tile_pool(name="dr", bufs=1, space="DRAM"))

    # ---- persistent SBUF buffers ----
    xt_sb = pool.tile([P, EC, L], BF16, tag="xbuf")
    wqkv_sb = pool.tile([P, EC, 3 * A], BF16, tag="wqkv")
    wout_sb = pool.tile([P, EC, E], BF16, tag="wout")
    cos_sb = pool.tile([P, L], BF16, tag="costab")
    sin_sb = pool.tile([P, L], BF16, tag="sintab")
    qk_ring = pool.tile([P, 2, 2, L], BF16, tag="qkring")  # [ring, q/k, L]
    vaug_sb = pool.tile([P, LKC, HC, 1 + D], BF16, tag="vaug")
    ones_sb = pool.tile([65, 64], BF16, tag="onesw")

    # ---- input DMAs, split so early matmuls unblock early ----
    xt_ap = xT.ap().rearrange("(c p) l -> p c l", p=P)
    wq_ap = wqkv.ap().rearrange("(c p) n -> p c n", p=P)
    nc.sync.dma_start(cos_sb[:], cosT.ap())
    nc.sync.dma_start(sin_sb[:], sinT.ap())
    nc.sync.dma_start(wqkv_sb[:, :, 0:A], wq_ap[:, :, 0:A])
    nc.sync.dma_start(xt_sb[:, :, 0:512], xt_ap[:, :, 0:512])
    nc.sync.dma_start(wqkv_sb[:, :, A : 2 * A], wq_ap[:, :, A : 2 * A])
    for piece in range(1, LT):
        sl = slice(piece * 512, (piece + 1) * 512)
        nc.sync.dma_start(xt_sb[:, :, sl], xt_ap[:, :, sl])
    nc.sync.dma_start(wqkv_sb[:, :, 2 * A : 3 * A], wq_ap[:, :, 2 * A : 3 * A])
    nc.sync.dma_start(wout_sb[:], wout.ap().rearrange("(c p) n -> p c n", p=P))
    nc.vector.memset(vaug_sb[:, :, :, D : D + 1], 1.0)
    nc.vector.memset(ones_sb[64:65, :], 1.0)

    # ---- PE work units (spliced between attention groups) ----
    def qk_unit(p, lt, qkk):
        """One [128, 512] tile of Q^T (qkk=0) or K^T (qkk=1) + RoPE."""
        ps = psum.tile([P, 512], F32, tag="spl", bufs=2, name=f"qps{p}{lt}{qkk}")
        wcol = qkk * A + p * P
        for e in range(EC):
            nc.tensor.matmul(
                ps[:],
                lhsT=wqkv_sb[:, e, wcol : wcol + P],
                rhs=xt_sb[:, e, lt * 512 : (lt + 1) * 512],
                start=(e == 0),
                stop=(e == EC - 1),
            )
        Lsl = slice(lt * 512, (lt + 1) * 512)
        qs = work.tile([P, 512], BF16, tag="qs", bufs=3, name=f"qs{p}{lt}{qkk}")
        nc.vector.tensor_copy(out=qs[:], in_=ps[:])
        w = work.tile([P, 512], BF16, tag="w", bufs=3, name=f"w{p}{lt}{qkk}")
        t = work.tile([P, 512], BF16, tag="t", bufs=3, name=f"t{p}{lt}{qkk}")
        nc.gpsimd.tensor_mul(w[:], qs[:], sin_sb[:, Lsl])
        nc.gpsimd.tensor_mul(t[:], qs[:], cos_sb[:, Lsl])
        wsw = work.tile([P, 512], BF16, tag="wsw", bufs=3, name=f"ws{p}{lt}{qkk}")
        for blk in range(4):
            sb = blk ^ 1  # swap 32-row blocks pairwise
            nc.sync.dma_start(
                wsw[blk * 32 : (blk + 1) * 32, :], w[sb * 32 : (sb + 1) * 32, :]
            )
        nc.vector.tensor_add(qk_ring[:, p % 2, qkk, Lsl], t[:], wsw[:])
        if taps is not None and (p, lt, qkk) == (1, 0, 0):
            nc.sync.dma_start(taps["qs1"].ap(), qs[:])
            nc.sync.dma_start(taps["w1"].ap(), w[:])
            nc.sync.dma_start(taps["t1"].ap(), t[:])
            nc.sync.dma_start(taps["wsw1"].ap(), wsw[:])

    def v_unit(g, u):
        """V for head quad g (heads 4g..4g+3), L-chunks 2u, 2u+1."""
        ps = psum.tile([P, 512], F32, tag="spl", bufs=2, name=f"vps{g}{u}")
        vcol = 2 * A + g * 4 * D
        for i in range(2):
            lc = 2 * u + i
            for e in range(EC):
                nc.tensor.matmul(
                    ps[:, i * 256 : (i + 1) * 256],
                    lhsT=xt_sb[:, e, lc * P : (lc + 1) * P],
                    rhs=wqkv_sb[:, e, vcol : vcol + 4 * D],
                    start=(e == 0),
                    stop=(e == EC - 1),
                )
        nc.vector.tensor_copy(
            out=vaug_sb[:, 2 * u : 2 * u + 2, 4 * g : 4 * g + 4, 0:D],
            in_=ps[:].rearrange("p (t h d) -> p t h d", t=2, h=4),
        )

    def qk_units(p):
        return [functools.partial(qk_unit, p, lt, qkk)
                for lt in range(LT) for qkk in (0, 1)]

    def v_units(g):
        return [functools.partial(v_unit, g, u) for u in range(LKC // 2)]

    # prefix: QK(pair 0) + V(heads 0..3) interleaved
    pre = []
    q0, v0 = qk_units(0), v_units(0)
    for i in range(max(len(q0), len(v0))):
        if i < len(q0):
            pre.append(q0[i])
        if i < len(v0):
            pre.append(v0[i])
    for unit in pre:
        unit()

    # splice plans: QK(p+1) during attn(p); V(heads 4..7) over attn(0..1)
    v1 = v_units(1)
    half = len(v1) // 2
    plan = {0: qk_units(1) + v1[:half],
            1: v1[half:] + qk_units(2),
            2: qk_units(3),
            3: []}

    # ---- exchange staging: one AllGather per lq tile ----
    # AG_j gathers both cores' [rec-den | raw O] rows for lq tile j.
    # cc_out is grouped by k = j % TH so each tail step k reads one tensor
    # at dynamic slot j // TH = core parity (even core: tiles 0..TH-1).
    pid = nc.gpsimd.partition_id()
    parity = pid % 2
    cc_in = [
        dram.tile([HC * 65, 512], BF16, tag=f"ccin{j}", bufs=1, name=f"ccin{j}")
        for j in range(LT)
    ]
    cc_out = [
        dram.tile([2, 2, HC * 65, 512], BF16, tag=f"ccout{k}", bufs=1,
                  name=f"ccout{k}")
        for k in range(TH)
    ]

    # ---- attention ----
    for p in range(NPAIR):
        r = p % 2
        for lq in range(LT):
            Lq = slice(lq * 512, (lq + 1) * 512)
            ot0 = psum.tile([65, 512], F32, tag="ot", bufs=2, name=f"ot0_{p}{lq}")
            ot1 = psum.tile([65, 512], F32, tag="ot", bufs=2, name=f"ot1_{p}{lq}")
            ots = (ot0, ot1)
            # splice schedule for this iteration
            todo = plan[p]
            iters_left = LT - lq
            n_emit = (len(todo) + iters_left - 1) // iters_left if todo else 0
            burst = todo[:n_emit]
            plan[p] = todo[n_emit:]
            spl_at = {}
            for i, u in enumerate(burst):
                spl_at.setdefault(1 + (i * (LKC - 2)) // max(1, len(burst)), []).append(u)
            for lk in range(LKC):
                ps = psum.tile([P, 1024], F32, tag="sc", bufs=2, name=f"sc{p}{lq}{lk}")
                for hh in (0, 1):
                    nc.tensor.matmul(
                        ps[:, hh * 512 : (hh + 1) * 512],
                        lhsT=qk_ring[hh * 64 : (hh + 1) * 64, r, 1, lk * P : (lk + 1) * P],
                        rhs=qk_ring[hh * 64 : (hh + 1) * 64, r, 0, Lq],
                        start=True,
                        stop=True,
                    )
                at = work.tile([P, 1024], BF16, tag="at", bufs=6, name=f"at{p}{lq}{lk}")
                nc.scalar.activation(at[:], ps[:], Exp, scale=scale)
                for hh in (0, 1):
                    nc.tensor.matmul(
                        ots[hh][:],
                        lhsT=vaug_sb[:, lk, 2 * p + hh, :],
                        rhs=at[:, hh * 512 : (hh + 1) * 512],
                        start=(lk == 0),
                        stop=(lk == LKC - 1),
                    )
                for u in spl_at.get(lk, ()):
                    u()
            # evacuate: row 0 -> reciprocal of denominator, rows 1..64 raw O
            for hh in (0, 1):
                rot = work.tile([65, 512], BF16, tag="rot", bufs=4, name=f"ro{p}{lq}{hh}")
                nc.vector.tensor_copy(out=rot[64:65, :], in_=ots[hh][64:65, :])
                nc.vector.tensor_copy(out=rot[0:64, :], in_=ots[hh][0:64, :])
                hrow = (2 * p + hh) * 65
                nc.sync.dma_start(cc_in[lq][hrow : hrow + 65, :], rot[:])
            if p == NPAIR - 1:
                ko, so = lq % TH, lq // TH
                if use_collective:
                    nc.gpsimd.collective_compute(
                        "AllGather",
                        mybir.AluOpType.bypass,
                        replica_groups=[[2 * i, 2 * i + 1] for i in range(N_CORES // 2)],
                        ins=[cc_in[lq][:].opt()],
                        outs=[cc_out[ko][so].opt()],
                    )
                else:  # timing-analysis build: stand-in copies, no collective
                    nc.sync.dma_start(cc_out[ko][so, 0], cc_in[lq][:])
                    nc.sync.dma_start(cc_out[ko][so, 1], cc_in[lq][:])

    # ---- receive, normalize, assemble, project (my L-half only) ----
    ocat = pool.tile([P, EC, L], BF16, tag="xbuf")  # aliases xt (dead by now)
    for k in range(TH):
        oc = work.tile([65, 2, HC, 512], BF16, tag="oc", bufs=1, name=f"oc{k}")
        for s in range(2):
            nc.gpsimd.dma_start(
                oc[:, s],
                cc_out[k][bass.ds(parity, 1), s].rearrange(
                    "a (h r) f -> (a r) h f", r=65
                ),
            )
        if taps is not None and k == 0:
            nc.sync.dma_start(taps["ccin0"].ap(), cc_in[0][:])
            nc.sync.dma_start(taps["oc0"].ap(), oc[:])
        onrm = work.tile([65, 2, HC, 512], BF16, tag="onrm", bufs=1, name=f"on{k}")
        Ln = mybir.ActivationFunctionType.Ln
        for s in range(2):
            lnt = work.tile([65, HC * 512], F32, tag="lnt", bufs=1, name=f"ln{k}{s}")
            nc.scalar.activation(lnt[64:65, :], oc[64:65, s, :, :], Ln)
            nc.scalar.activation(oc[64:65, s, :, :], lnt[64:65, :], Exp, scale=-1.0)
        for s in range(2):
            for hl in range(HC):
                rb = psum.tile([64, 512], F32, tag="ot", bufs=2, name=f"rb{k}{s}{hl}")
                nc.tensor.matmul(
                    rb[:], lhsT=ones_sb[64:65, :], rhs=oc[64:65, s, hl, :],
                    start=True, stop=True,
                )
                nc.vector.tensor_mul(
                    onrm[0:64, s, hl, :], oc[0:64, s, hl, :], rb[0:64, :]
                )
        if taps is not None and k == 0:
            nc.sync.dma_start(taps["onrm0"].ap(), onrm[:])
        for e in range(EC):
            s, hl = (2 * e) // HC, (2 * e) % HC
            for half2 in range(2):
                nc.sync.dma_start(
                    ocat[half2 * 64 : (half2 + 1) * 64, e, k * 512 : (k + 1) * 512],
                    onrm[0:64, s, hl + half2, :],
                )
        for c128 in range(4):
            row = k * 512 + c128 * P
            for nhf in range(E // 512):
                ps = psum.tile([P, 512], F32, tag="spl", bufs=2, name=f"yp{k}{c128}{nhf}")
                for e in range(EC):
                    nc.tensor.matmul(
                        ps[:],
                        lhsT=ocat[:, e, row : row + P],
                        rhs=wout_sb[:, e, nhf * 512 : (nhf + 1) * 512],
                        start=(e == 0),
                        stop=(e == EC - 1),
                    )
                yt = work.tile([P, 512], F32, tag="yt", bufs=2, name=f"yt{k}{c128}{nhf}")
                nc.vector.tensor_copy(out=yt[:], in_=ps[:])
                nc.sync.dma_start(
                    y.ap()[row : row + P, nhf * 512 : (nhf + 1) * 512], yt[:]
                )

    if taps is not None:
        nc.sync.dma_start(taps["ocat"].ap().rearrange("(c p) l -> p c l", p=P),
                          ocat[:])
        nc.sync.dma_start(taps["vaug"].ap(), vaug_sb[:])
        nc.sync.dma_start(taps["qkr"].ap(), qk_ring[:])

    ctx.close()


@functools.lru_cache(maxsize=2)
def build_module(L=L_FULL, E=E_FULL, HC=H_FULL // 2, D=64, asserts=False,
                 use_collective=True, debug_taps=False):
    nc = bacc.Bacc(
        "TRN2",
        target_bir_lowering=False,
        debug=False,
        enable_asserts=asserts,
        num_devices=N_CORES,
    )
    A = HC * D
    xT = nc.dram_tensor("xT", [E, L], BF16, kind="ExternalInput")
    wqkv = nc.dram_tensor("wqkv", [E, 3 * A], BF16, kind="ExternalInput")
    wout = nc.dram_tensor("wout", [E, E], BF16, kind="ExternalInput")
    cosT = nc.dram_tensor("cosT", [128, L], BF16, kind="ExternalInput")
    sinT = nc.dram_tensor("sinT", [128, L], BF16, kind="ExternalInput")
    y = nc.dram_tensor("y", [L // 2, E], F32, kind="ExternalOutput")
    taps = None
    if debug_taps:
        taps = {
            "ccin0": nc.dram_tensor("ccin0_dbg", [HC * 65, 512], BF16, kind="ExternalOutput"),
            "oc0": nc.dram_tensor("oc0_dbg", [65, 2, HC, 512], BF16, kind="ExternalOutput"),
            "onrm0": nc.dram_tensor("onrm0_dbg", [65, 2, HC, 512], BF16, kind="ExternalOutput"),
            "ocat": nc.dram_tensor("ocat_dbg", [E, L], BF16, kind="ExternalOutput"),
            "vaug": nc.dram_tensor("vaug_dbg", [128, L // 128, HC, D + 1], BF16, kind="ExternalOutput"),
            "qkr": nc.dram_tensor("qkr_dbg", [128, 2, 2, L], BF16, kind="ExternalOutput"),
            "qs1": nc.dram_tensor("qs1_dbg", [128, 512], BF16, kind="ExternalOutput"),
            "w1": nc.dram_tensor("w1_dbg", [128, 512], BF16, kind="ExternalOutput"),
            "t1": nc.dram_tensor("t1_dbg", [128, 512], BF16, kind="ExternalOutput"),
            "wsw1": nc.dram_tensor("wsw1_dbg", [128, 512], BF16, kind="ExternalOutput"),
        }
    with tile.TileContext(nc) as tc:
        _emit(tc, nc, xT, wqkv, wout, cosT, sinT, y, L, E, HC, D,
              use_collective=use_collective, taps=taps)
    nc.compile()
    return nc


def _rope_tables(L, D):
    """cos/sin tables in the de-interleaved 32-row layout, stacked x4.

    Row p (p in [0,32)): frequency p (covers original dims 2p / 2p+1).
    sin is pre-signed for the post-swap add: blocks [+s, -s, +s, -s].
    """
    half = D // 2
    inv_freq = 1.0 / (ROPE_THETA ** (np.arange(0, D, 2, dtype=np.float64) / D))
    freqs = np.arange(L, dtype=np.float64)[None, :] * inv_freq[:, None]  # [32, L]
    cos32 = np.cos(freqs)
    sin32 = np.sin(freqs)
    bf = ml_dtypes.bfloat16
    cos = np.tile(cos32, (128 // half, 1)).astype(bf)
    sin_block = np.concatenate([sin32, -sin32], axis=0)  # [64, L]
    sin = np.tile(sin_block, (2, 1)).astype(bf)
    return cos, sin


def _deint_cols(base, h, D):
    """Column indices of head h (offset base), even dims then odd dims."""
    cols = base + h * D + np.arange(D)
    return np.concatenate([cols[0::2], cols[1::2]])


def make_core_inputs(x, w_qkv, w_out, H=H_FULL, D=64):
    """Per-core input dicts from the full (unsharded) fp32 inputs."""
    Bv, L, E = x.shape
    HC = H // (N_CORES // Bv)
    A_full = H * D
    bf = ml_dtypes.bfloat16
    cos, sin = _rope_tables(L, D)
    wout_bf = np.ascontiguousarray(w_out).astype(bf)
    in_maps = []
    for c in range(N_CORES):
        b, g = c // 2, c % 2
        xT = np.ascontiguousarray(x[b].T).astype(bf)
        qcols = []
        kcols = []
        vcols = []
        for p in range(HC // 2):
            for hh in range(2):
                h = g * HC + 2 * p + hh
                qcols.append(_deint_cols(0, h, D))
                kcols.append(_deint_cols(A_full, h, D))
        for hl in range(HC):
            h = g * HC + hl
            vcols.append(2 * A_full + h * D + np.arange(D))
        cols = np.concatenate(qcols + kcols + vcols)
        wqkv_c = np.ascontiguousarray(w_qkv[:, cols]).astype(bf)
        in_maps.append(
            {
                "xT": xT,
                "wqkv": wqkv_c,
                "wout": wout_bf,
                "cosT": cos[:, :L].copy(),
                "sinT": sin[:, :L].copy(),
            }
        )
    return in_maps


def kernel(x, w_qkv, w_out):
    x = np.asarray(x)
    w_qkv = np.asarray(w_qkv)
    w_out = np.asarray(w_out)
    Bv, L, E = x.shape
    nc = build_module(L=L, E=E)
    in_maps = make_core_inputs(x, w_qkv, w_out)
    res = run_bass_kernel_spmd(nc, in_maps, core_ids=list(range(N_CORES)))
    out = np.empty((Bv, L, E), dtype=np.float32)
    for b in range(Bv):
        out[b, : L // 2] = res.results[2 * b]["y"]
        out[b, L // 2 :] = res.results[2 * b + 1]["y"]
    return out
